# revision 13
# baseline (speedup 1.0000x reference)
"""Trainium2 (8-core SPMD) kernel for the ActorCriticTensorNet MPS head.

reference:
    env0 = einsum('e,eoij->oij', x[0], mps[0])
    for a in 1..63: env = sigmoid(env @ einsum('e,eoij->oij', x[a], mps[a]))
    out = einsum('oii->o', env)

Strategy: the per-agent contractions mat[a] = x[a] . mps[a] are independent;
only the 63-step sigmoid chain is sequential.  Agents are sharded *strided*
(core c holds agents c, c+8, ..., c+56) so that after each core finishes its
g-th local agent, one small AllGather delivers the contiguous global block
[8g, 8g+8) of mats to every core and the sequential chain advances 8 steps —
the chain and the per-group AllGathers overlap the next group's tensor
streaming.  mps/x are converted to bf16 on the host (halves HBM traffic;
PSUM accumulation stays fp32).

The chain keeps env transposed per o-block: with env_T[o][k,i] = env[o][i,k],
    new_env_T[o] = sigmoid( matmul(lhsT=mat[o], rhs=env_T[o]) )
so no per-step transposes are needed.  mats are stored in DRAM layout
d = k*256 + j*8 + o, which makes the phase-1 weight views, the psum->stage
permute, the DRAM stores, the chain loads and the chain weight views all
single-free-dim APs / contiguous DMAs.  The two o-halves of the chain run as
independent pipelines to halve its serial latency.
"""

import numpy as np

A, E, O, C = 64, 256, 8, 32
F = O * C * C  # 8192
N_CORES = 8
AL = A // N_CORES  # agents per core
HO = O // 2

_CACHE = {}


def _build(debug_out=False):
    from concourse import bacc, mybir, tile
    from concourse.masks import make_identity

    F32 = mybir.dt.float32
    BF16 = mybir.dt.bfloat16
    SIG = mybir.ActivationFunctionType.Sigmoid
    nc = bacc.Bacc(
        "TRN2", target_bir_lowering=False, debug=False, num_devices=N_CORES
    )
    x_d = nc.dram_tensor("inputs", [AL, E], BF16, kind="ExternalInput")
    mps_d = nc.dram_tensor("mps", [AL, E, F], BF16, kind="ExternalInput")
    out_d = nc.dram_tensor("out", [O, 1], F32, kind="ExternalOutput")
    if debug_out:
        matdbg_d = nc.dram_tensor("matdbg", [A, F], BF16, kind="ExternalOutput")
        envdbg_d = nc.dram_tensor("envdbg", [C, 256], F32, kind="ExternalOutput")

    with tile.TileContext(nc) as tc:
        with (
            tc.tile_pool(name="dram", bufs=1, space="DRAM") as dram,
            tc.tile_pool(name="mps_pool", bufs=8) as mps_pool,
            tc.tile_pool(name="small", bufs=1) as small,
            tc.tile_pool(name="stage_pool", bufs=4) as stage_pool,
            tc.tile_pool(name="chain_pool", bufs=10) as chain_pool,
            tc.tile_pool(name="env_pool", bufs=2) as env_pool,
            tc.tile_pool(name="ps_mat", bufs=2, space="PSUM") as ps_mat,
            tc.tile_pool(name="ps_chain", bufs=2, space="PSUM") as ps_chain,
            tc.tile_pool(name="ps_misc", bufs=1, space="PSUM") as ps_misc,
        ):
            # AG over one local agent la concatenates the 8 ranks' mats,
            # which with strided sharding is the global block [8la, 8la+8).
            GPA = 2  # local agents gathered per AllGather
            mat_loc = dram.tile([AL, F], BF16)
            mat_alls = [
                dram.tile(
                    [N_CORES * GPA, F],
                    BF16,
                    addr_space="Shared",
                    name=f"mat_all{gr}",
                )
                for gr in range(AL // GPA)
            ]
            mat_all_vs = [
                m[:].rearrange("g (k r) -> g k r", k=C, r=256) for m in mat_alls
            ]

            # x columns: x_sb[e_lo, 2*a + e_hi] = x[a, e_hi*128 + e_lo]
            x_sb = small.tile([128, AL * 2], BF16)
            x_view = x_d[:].rearrange("a (eh el u) -> a eh el u", eh=2, el=128, u=1)
            for a in range(AL):
                for eh in range(2):
                    nc.sync.dma_start(
                        x_sb[:, 2 * a + eh : 2 * a + eh + 1], x_view[a, eh]
                    )

            ident = small.tile([C, C], BF16)
            make_identity(nc, ident[:])

            mps_view = mps_d[:].rearrange("a (eh el) f -> a eh el f", eh=2, el=128)
            mat_loc_v = mat_loc[:].rearrange("a (p b) -> a p b", p=128, b=64)

            envs = [None, None]

            def phase1_agent(a):
                # mat[a][o,k,j] = sum_e x[a,e] mps[a,e,o,k,j]
                # weight column b = o*8+jl reads f = o*1024 + 8*p + jl
                # (p = 4k + j_hi) -> stride-8 single-free-dim AP.
                # psum[p, b] = mat[o][k][j], k = p>>2, j = (p&3)*8 + jl.
                psum_a = ps_mat.tile([128, 64], F32, tag="psa")
                tvs = []
                for eh in range(2):
                    t = mps_pool.tile([128, F], BF16, tag="mps")
                    nc.sync.dma_start(t[:], mps_view[a, eh])
                    tvs.append(
                        t[:].rearrange("e (o p jl) -> e o p jl", o=O, p=128, jl=8)
                    )
                # start/stop pairs per psum column must be consecutive
                # (interleaved accumulation groups compute wrong results
                # on HW).
                for o in range(O):
                    for jl in range(8):
                        for eh in range(2):
                            nc.tensor.matmul(
                                psum_a[:, o * 8 + jl : o * 8 + jl + 1],
                                tvs[eh][:, o, :, jl],
                                x_sb[:, 2 * a + eh : 2 * a + eh + 1],
                                start=(eh == 0),
                                stop=(eh == 1),
                            )
                # DRAM layout d = k*256 + j*8 + o = 64p + (jl*8 + o):
                # permute columns (o,jl)->(jl,o) on the way out of PSUM.
                stage = stage_pool.tile([128, 64], BF16, tag="stage")
                nc.vector.tensor_copy(
                    stage[:].rearrange("p (jl o) -> p jl o", jl=8, o=8),
                    psum_a[:].rearrange("p (o jl) -> p jl o", o=8, jl=8),
                )
                nc.sync.dma_start(mat_loc_v[a], stage[:])

            def chain_step(g, init=False):
                sb_g = chain_pool.tile([C, 256], BF16, tag="chain")
                la, r = g // 8, g % 8
                nc.scalar.dma_start(
                    sb_g[:], mat_all_vs[la // GPA][r * GPA + (la % GPA)]
                )
                v = sb_g[:].rearrange("k (j o) -> k j o", j=C, o=O)
                for h in range(2):
                    # is_transpose matmuls require psum dtype == input dtype
                    ps_g = ps_chain.tile(
                        [C, HO * C], BF16 if init else F32, tag=f"cps{h}"
                    )
                    for oo in range(HO):
                        o = h * HO + oo
                        if init:
                            nc.tensor.transpose(
                                ps_g[:, oo * C : (oo + 1) * C],
                                v[:, :, o],
                                ident[:],
                            )
                        else:
                            nc.tensor.matmul(
                                ps_g[:, oo * C : (oo + 1) * C],
                                v[:, :, o],
                                envs[h][:, oo * C : (oo + 1) * C],
                                start=True,
                                stop=True,
                            )
                    env2 = env_pool.tile([C, HO * C], BF16, tag=f"env{h}")
                    if init:
                        nc.vector.tensor_copy(env2[:], ps_g[:])
                    else:
                        nc.scalar.activation(env2[:], ps_g[:], SIG)
                    envs[h] = env2

            for gr in range(AL // GPA):
                for la in range(gr * GPA, (gr + 1) * GPA):
                    phase1_agent(la)
                nc.gpsimd.collective_compute(
                    "AllGather",
                    mybir.AluOpType.bypass,
                    replica_groups=[list(range(N_CORES))],
                    ins=[mat_loc[gr * GPA : (gr + 1) * GPA, :].opt()],
                    outs=[mat_alls[gr].opt()],
                )
                for g in range(gr * GPA * 8, (gr + 1) * GPA * 8):
                    chain_step(g, init=(g == 0))

            if debug_out:
                dbg = small.tile([C, 256], F32)
                nc.vector.tensor_copy(dbg[:, 0 : HO * C], envs[0][:])
                nc.vector.tensor_copy(dbg[:, HO * C :], envs[1][:])
                nc.sync.dma_start(envdbg_d[:], dbg[:])
                for gr in range(AL // GPA):
                    nc.sync.dma_start(
                        matdbg_d[gr * GPA * 8 : (gr + 1) * GPA * 8, :],
                        mat_alls[gr][:],
                    )

            # ---- trace epilogue: out[o] = sum_k env_T[o][k, k] ----
            masked = small.tile([C, 256], BF16)
            red = small.tile([C, O], F32)
            for h in range(2):
                mh = masked[:, h * HO * C : (h + 1) * HO * C]
                nc.gpsimd.affine_select(
                    out=mh.rearrange("k (o i) -> k o i", o=HO, i=C),
                    in_=envs[h][:].rearrange("k (o i) -> k o i", o=HO, i=C),
                    compare_op=mybir.AluOpType.is_equal,
                    fill=0.0,
                    base=0,
                    pattern=[[0, HO], [1, C]],
                    channel_multiplier=-1,
                )
                nc.vector.tensor_reduce(
                    red[:, h * HO : (h + 1) * HO].rearrange(
                        "k (o u) -> k o u", u=1
                    ),
                    mh.rearrange("k (o i) -> k o i", o=HO, i=C),
                    axis=mybir.AxisListType.X,
                    op=mybir.AluOpType.add,
                )
            ones = small.tile([C, 1], F32)
            nc.gpsimd.memset(ones[:], 1.0)
            pt = ps_misc.tile([O, 1], F32)
            nc.tensor.matmul(pt[:], red[:], ones[:], start=True, stop=True)
            osb = small.tile([O, 1], F32)
            nc.vector.tensor_copy(osb[:], pt[:])
            nc.sync.dma_start(out_d[:], osb[:])

    nc.compile()
    return nc


def get_nc():
    if "nc" not in _CACHE:
        _CACHE["nc"] = _build()
    return _CACHE["nc"]


def make_in_maps(inputs, mps):
    import ml_dtypes

    inputs = np.asarray(inputs, dtype=np.float32).astype(ml_dtypes.bfloat16)
    mps = (
        np.asarray(mps, dtype=np.float32)
        .reshape(A, E, F)
        .astype(ml_dtypes.bfloat16)
    )
    in_maps = []
    for c in range(N_CORES):
        # strided sharding: core c holds global agents c, c+8, ..., c+56
        in_maps.append(
            {
                "inputs": np.ascontiguousarray(inputs[c::N_CORES]),
                "mps": np.ascontiguousarray(mps[c::N_CORES]),
            }
        )
    return in_maps


def kernel(inputs, mps):
    from concourse.bass_utils import run_bass_kernel_spmd

    nc = get_nc()
    res = run_bass_kernel_spmd(
        nc, make_in_maps(inputs, mps), core_ids=list(range(N_CORES))
    )
    return res.results[0]["out"].reshape(O).astype(np.float32)


# revision 14
# speedup vs baseline: 1.3394x; 1.3394x over previous
"""Trainium2 (8-core SPMD) kernel for the ActorCriticTensorNet MPS head.

reference:
    env0 = einsum('e,eoij->oij', x[0], mps[0])
    for a in 1..63: env = sigmoid(env @ einsum('e,eoij->oij', x[a], mps[a]))
    out = einsum('oii->o', env)

Strategy: the computation factorizes perfectly over the output channel o —
the per-agent contractions mat[a][o] and the 63-step sigmoid chain for
channel o never touch any other channel; the channels only meet in the final
trace vector.  So shard by o: core c receives mps[:, :, c, :, :] (32 MB in
bf16) plus the full (tiny) x, computes all 64 mat[g][c] slices locally,
runs its own o=c chain locally, and writes the single scalar out[c]; the
host assembles the 8 scalars.  Zero inter-core communication, and the chain
consumes mats as phase 1 produces them, so its serial latency hides entirely
under the tensor streaming.

Phase-1 layout (per 8-agent block): weight column (gi, jl) reads the
stride-8 single-free-dim AP f = gi*1024 + 8p + jl (p = 4k + j_hi), giving
psum[p, gi*8+jl] = mat[gi][k, j] (k = p>>2, j = (p&3)*8 + jl).  The
psum->stage copy permutes columns (gi,jl)->(jl,gi) so the DRAM block is
d = 64p + jl*8 + gi = k*256 + j_hi*64 + jl*8 + gi: the store, the one
chain-block load (32 x 256, 512B runs) and the per-agent chain weight views
(stride-8, merging to a single free dim) are all clean APs.

Host-side prep packs shards as mps[c] -> (2, 128, 64*1024) [e-chunk, e_low,
(agent, f)] and x -> (2, 128, 64), so every device DMA is contiguous 2 MB.
All inputs are converted to bf16 on the host (PSUM accumulation is fp32;
measured end-to-end relative error ~6e-4 vs the fp32 reference).
"""

import numpy as np

A, E, O, C = 64, 256, 8, 32
FO = C * C  # per-o mat size: 1024
N_CORES = 8
BLK = 8  # agents per phase-1 block
NBLK = A // BLK

_CACHE = {}


def _build(debug_out=False):
    from concourse import bacc, mybir, tile
    from concourse.masks import make_identity

    F32 = mybir.dt.float32
    BF16 = mybir.dt.bfloat16
    SIG = mybir.ActivationFunctionType.Sigmoid
    nc = bacc.Bacc(
        "TRN2", target_bir_lowering=False, debug=False, num_devices=N_CORES
    )
    x_d = nc.dram_tensor("inputs", [2, 128, A], BF16, kind="ExternalInput")
    mps_d = nc.dram_tensor(
        "mps", [2, 128, A * FO], BF16, kind="ExternalInput"
    )
    out_d = nc.dram_tensor("out", [1, 1], F32, kind="ExternalOutput")
    if debug_out:
        matdbg_d = nc.dram_tensor("matdbg", [A, FO], BF16, kind="ExternalOutput")
        envdbg_d = nc.dram_tensor("envdbg", [C, C], F32, kind="ExternalOutput")

    with tile.TileContext(nc) as tc:
        with (
            tc.tile_pool(name="dram", bufs=1, space="DRAM") as dram,
            tc.tile_pool(name="mps_pool", bufs=6) as mps_pool,
            tc.tile_pool(name="small", bufs=1) as small,
            tc.tile_pool(name="stage_pool", bufs=4) as stage_pool,
            tc.tile_pool(name="chain_pool", bufs=4) as chain_pool,
            tc.tile_pool(name="env_pool", bufs=2) as env_pool,
            tc.tile_pool(name="ps_mat", bufs=3, space="PSUM") as ps_mat,
            tc.tile_pool(name="ps_chain", bufs=2, space="PSUM") as ps_chain,
            tc.tile_pool(name="ps_misc", bufs=1, space="PSUM") as ps_misc,
        ):
            mat_dram = dram.tile([NBLK, 128 * 64], BF16)

            # x_sb[e_lo, eh*64 + g] = x[g, eh*128 + e_lo]
            x_sb = small.tile([128, 2 * A], BF16)
            for eh in range(2):
                nc.sync.dma_start(x_sb[:, eh * A : (eh + 1) * A], x_d[eh])

            ident = small.tile([C, C], BF16)
            make_identity(nc, ident[:])

            envs = [None]

            def phase1_block(blk):
                psum_b = ps_mat.tile([128, 64], F32, tag="psa")
                tvs = []
                for eh in range(2):
                    t = mps_pool.tile([128, BLK * FO], BF16, tag="mps")
                    nc.sync.dma_start(
                        t[:], mps_d[eh, :, blk * BLK * FO : (blk + 1) * BLK * FO]
                    )
                    tvs.append(
                        t[:].rearrange(
                            "e (gi p jl) -> e gi p jl", gi=BLK, p=128, jl=8
                        )
                    )
                # start/stop pairs per psum column must be consecutive
                # (interleaved accumulation groups are wrong on HW).
                for gi in range(BLK):
                    g = blk * BLK + gi
                    for jl in range(8):
                        for eh in range(2):
                            nc.tensor.matmul(
                                psum_b[:, gi * 8 + jl : gi * 8 + jl + 1],
                                tvs[eh][:, gi, :, jl],
                                x_sb[:, eh * A + g : eh * A + g + 1],
                                start=(eh == 0),
                                stop=(eh == 1),
                            )
                stage = stage_pool.tile([128, 64], BF16, tag="stage")
                nc.vector.tensor_copy(
                    stage[:].rearrange("p (jl gi) -> p jl gi", jl=8, gi=8),
                    psum_b[:].rearrange("p (gi jl) -> p jl gi", gi=8, jl=8),
                )
                nc.sync.dma_start(mat_dram[blk : blk + 1, :], stage[:])

            def chain_block(blk):
                # one clean (32, 256) load per block; agent gi's weight is
                # the stride-8 view [jh: 64x4][jl: 8x8] at offset gi, which
                # opt-merges into a single free dim.
                cb = chain_pool.tile([C, 256], BF16, tag="chain")
                nc.scalar.dma_start(
                    cb[:],
                    mat_dram[blk : blk + 1, :].rearrange(
                        "u (k r) -> (u k) r", k=C, r=256
                    ),
                )
                cv = cb[:].rearrange("k (jh jl gi) -> k jh jl gi", jh=4, jl=8, gi=8)
                for gi in range(BLK):
                    init = blk == 0 and gi == 0
                    ps_g = ps_chain.tile([C, C], BF16 if init else F32, tag="cps")
                    if init:
                        nc.tensor.transpose(ps_g[:], cv[:, :, :, gi], ident[:])
                    else:
                        nc.tensor.matmul(
                            ps_g[:],
                            cv[:, :, :, gi],
                            envs[0][:],
                            start=True,
                            stop=True,
                        )
                    env2 = env_pool.tile([C, C], BF16, tag="env")
                    if init:
                        nc.vector.tensor_copy(env2[:], ps_g[:])
                    else:
                        nc.scalar.activation(env2[:], ps_g[:], SIG)
                    envs[0] = env2

            for blk in range(NBLK):
                phase1_block(blk)
                chain_block(blk)

            env = envs[0]
            if debug_out:
                dbg = small.tile([C, C], F32)
                nc.vector.tensor_copy(dbg[:], env[:])
                nc.sync.dma_start(envdbg_d[:], dbg[:])
                nc.sync.dma_start(
                    matdbg_d[:],
                    mat_dram[:].rearrange("b (p c2) -> (b p) c2", p=128, c2=64),
                )

            # ---- trace epilogue: out = sum_k env_T[k, k] ----
            masked = small.tile([C, C], BF16)
            nc.gpsimd.affine_select(
                out=masked[:],
                in_=env[:],
                compare_op=mybir.AluOpType.is_equal,
                fill=0.0,
                base=0,
                pattern=[[1, C]],
                channel_multiplier=-1,
            )
            red = small.tile([C, 1], F32)
            nc.vector.tensor_reduce(
                red[:],
                masked[:],
                axis=mybir.AxisListType.X,
                op=mybir.AluOpType.add,
            )
            ones = small.tile([C, 1], F32)
            nc.gpsimd.memset(ones[:], 1.0)
            pt = ps_misc.tile([1, 1], F32)
            nc.tensor.matmul(pt[:], red[:], ones[:], start=True, stop=True)
            osb = small.tile([1, 1], F32)
            nc.vector.tensor_copy(osb[:], pt[:])
            nc.sync.dma_start(out_d[:], osb[:])

    nc.compile()
    return nc


def get_nc():
    if "nc" not in _CACHE:
        _CACHE["nc"] = _build()
    return _CACHE["nc"]


def make_in_maps(inputs, mps):
    import ml_dtypes

    x = np.asarray(inputs, dtype=np.float32).astype(ml_dtypes.bfloat16)
    mps = np.asarray(mps, dtype=np.float32).reshape(A, E, O, FO)
    # x packed as [e_chunk, e_low, agent]
    x_pack = np.ascontiguousarray(x.reshape(A, 2, 128).transpose(1, 2, 0))
    in_maps = []
    for c in range(N_CORES):
        m = mps[:, :, c, :].astype(ml_dtypes.bfloat16)  # (A, E, FO)
        m = m.reshape(A, 2, 128, FO).transpose(1, 2, 0, 3)  # (2, 128, A, FO)
        in_maps.append(
            {
                "inputs": x_pack,
                "mps": np.ascontiguousarray(m).reshape(2, 128, A * FO),
            }
        )
    return in_maps


def kernel(inputs, mps):
    from concourse.bass_utils import run_bass_kernel_spmd

    nc = get_nc()
    res = run_bass_kernel_spmd(
        nc, make_in_maps(inputs, mps), core_ids=list(range(N_CORES))
    )
    return np.array(
        [res.results[c]["out"][0, 0] for c in range(N_CORES)], dtype=np.float32
    )


# revision 15
# speedup vs baseline: 1.9389x; 1.4476x over previous
"""Trainium2 (8-core SPMD) kernel for the ActorCriticTensorNet MPS head.

reference:
    env0 = einsum('e,eoij->oij', x[0], mps[0])
    for a in 1..63: env = sigmoid(env @ einsum('e,eoij->oij', x[a], mps[a]))
    out = einsum('oii->o', env)

Strategy: the computation factorizes perfectly over the output channel o —
the per-agent contractions mat[a][o] and the 63-step sigmoid chain for
channel o never touch any other channel; the channels only meet in the final
trace vector.  So shard by o: core c receives mps[:, :, c, :, :] (32 MB in
bf16) plus the full (tiny) x, computes all 64 mat[g][c] slices locally,
runs its own o=c chain locally, and writes the single scalar out[c]; the
host assembles the 8 scalars.  Zero inter-core communication, and the chain
consumes mats as phase 1 produces them, so its serial latency hides entirely
under the tensor streaming.

Phase-1 layout (per 8-agent block): weight column (gi, jl) reads the
stride-8 single-free-dim AP f = gi*1024 + 8p + jl (p = 4k + j_hi), giving
psum[p, gi*8+jl] = mat[gi][k, j] (k = p>>2, j = (p&3)*8 + jl).  The
psum->stage copy permutes columns (gi,jl)->(jl,gi) so the DRAM block is
d = 64p + jl*8 + gi = k*256 + j_hi*64 + jl*8 + gi: the store, the one
chain-block load (32 x 256, 512B runs) and the per-agent chain weight views
(stride-8, merging to a single free dim) are all clean APs.

Host-side prep packs shards as mps[c] -> (2, 128, 64*1024) [e-chunk, e_low,
(agent, f)] and x -> (2, 128, 64), so every device DMA is contiguous 2 MB.
All inputs are converted to bf16 on the host (PSUM accumulation is fp32;
measured end-to-end relative error ~6e-4 vs the fp32 reference).
"""

import numpy as np

A, E, O, C = 64, 256, 8, 32
FO = C * C  # per-o mat size: 1024
N_CORES = 8
BLK = 8  # agents per phase-1 block
NBLK = A // BLK

_CACHE = {}


def _build(debug_out=False):
    from concourse import bacc, mybir, tile
    from concourse.masks import make_identity

    F32 = mybir.dt.float32
    BF16 = mybir.dt.bfloat16
    SIG = mybir.ActivationFunctionType.Sigmoid
    nc = bacc.Bacc(
        "TRN2", target_bir_lowering=False, debug=False, num_devices=N_CORES
    )
    x_d = nc.dram_tensor("inputs", [2, 128, A], BF16, kind="ExternalInput")
    mps_d = nc.dram_tensor(
        "mps", [2, 128, A * FO], BF16, kind="ExternalInput"
    )
    out_d = nc.dram_tensor("out", [1, 1], F32, kind="ExternalOutput")
    if debug_out:
        matdbg_d = nc.dram_tensor("matdbg", [A, FO], BF16, kind="ExternalOutput")
        envdbg_d = nc.dram_tensor("envdbg", [C, C], F32, kind="ExternalOutput")

    with tile.TileContext(nc) as tc:
        with (
            tc.tile_pool(name="dram", bufs=1, space="DRAM") as dram,
            tc.tile_pool(name="mps_pool", bufs=9) as mps_pool,
            tc.tile_pool(name="small", bufs=1) as small,
            tc.tile_pool(name="stage_pool", bufs=4) as stage_pool,
            tc.tile_pool(name="chain_pool", bufs=4) as chain_pool,
            tc.tile_pool(name="env_pool", bufs=2) as env_pool,
            tc.tile_pool(name="ps_mat", bufs=3, space="PSUM") as ps_mat,
            tc.tile_pool(name="ps_chain", bufs=2, space="PSUM") as ps_chain,
            tc.tile_pool(name="ps_misc", bufs=1, space="PSUM") as ps_misc,
        ):
            mat_dram = dram.tile([NBLK, 128 * 64], BF16)

            # x_sb[e_lo, eh*64 + g] = x[g, eh*128 + e_lo]
            x_sb = small.tile([128, 2 * A], BF16)
            for eh in range(2):
                nc.sync.dma_start(x_sb[:, eh * A : (eh + 1) * A], x_d[eh])

            ident = small.tile([C, C], BF16)
            make_identity(nc, ident[:])

            envs = [None]

            def phase1_block(blk):
                psum_b = ps_mat.tile([128, 64], F32, tag="psa")
                tvs = []
                for eh in range(2):
                    t = mps_pool.tile([128, BLK * FO], BF16, tag="mps")
                    nc.sync.dma_start(
                        t[:], mps_d[eh, :, blk * BLK * FO : (blk + 1) * BLK * FO]
                    )
                    tvs.append(
                        t[:].rearrange(
                            "e (gi p jl) -> e gi p jl", gi=BLK, p=128, jl=8
                        )
                    )
                # start/stop pairs per psum column must be consecutive
                # (interleaved accumulation groups are wrong on HW).
                for gi in range(BLK):
                    g = blk * BLK + gi
                    for jl in range(8):
                        for eh in range(2):
                            nc.tensor.matmul(
                                psum_b[:, gi * 8 + jl : gi * 8 + jl + 1],
                                tvs[eh][:, gi, :, jl],
                                x_sb[:, eh * A + g : eh * A + g + 1],
                                start=(eh == 0),
                                stop=(eh == 1),
                            )
                stage = stage_pool.tile([128, 64], BF16, tag="stage")
                nc.vector.tensor_copy(
                    stage[:].rearrange("p (jl gi) -> p jl gi", jl=8, gi=8),
                    psum_b[:].rearrange("p (gi jl) -> p jl gi", gi=8, jl=8),
                )
                nc.gpsimd.dma_start(mat_dram[blk : blk + 1, :], stage[:])

            def chain_block(blk):
                # one clean (32, 256) load per block; agent gi's weight is
                # the stride-8 view [jh: 64x4][jl: 8x8] at offset gi, which
                # opt-merges into a single free dim.
                cb = chain_pool.tile([C, 256], BF16, tag="chain")
                nc.scalar.dma_start(
                    cb[:],
                    mat_dram[blk : blk + 1, :].rearrange(
                        "u (k r) -> (u k) r", k=C, r=256
                    ),
                )
                cv = cb[:].rearrange("k (jh jl gi) -> k jh jl gi", jh=4, jl=8, gi=8)
                for gi in range(BLK):
                    init = blk == 0 and gi == 0
                    ps_g = ps_chain.tile([C, C], BF16 if init else F32, tag="cps")
                    if init:
                        nc.tensor.transpose(ps_g[:], cv[:, :, :, gi], ident[:])
                    else:
                        nc.tensor.matmul(
                            ps_g[:],
                            cv[:, :, :, gi],
                            envs[0][:],
                            start=True,
                            stop=True,
                        )
                    env2 = env_pool.tile([C, C], BF16, tag="env")
                    if init:
                        nc.vector.tensor_copy(env2[:], ps_g[:])
                    else:
                        nc.scalar.activation(env2[:], ps_g[:], SIG)
                    envs[0] = env2

            for blk in range(NBLK):
                phase1_block(blk)
                chain_block(blk)

            env = envs[0]
            if debug_out:
                dbg = small.tile([C, C], F32)
                nc.vector.tensor_copy(dbg[:], env[:])
                nc.sync.dma_start(envdbg_d[:], dbg[:])
                nc.sync.dma_start(
                    matdbg_d[:],
                    mat_dram[:].rearrange("b (p c2) -> (b p) c2", p=128, c2=64),
                )

            # ---- trace epilogue: out = sum_k env_T[k, k] ----
            masked = small.tile([C, C], BF16)
            nc.gpsimd.affine_select(
                out=masked[:],
                in_=env[:],
                compare_op=mybir.AluOpType.is_equal,
                fill=0.0,
                base=0,
                pattern=[[1, C]],
                channel_multiplier=-1,
            )
            red = small.tile([C, 1], F32)
            nc.vector.tensor_reduce(
                red[:],
                masked[:],
                axis=mybir.AxisListType.X,
                op=mybir.AluOpType.add,
            )
            ones = small.tile([C, 1], F32)
            nc.gpsimd.memset(ones[:], 1.0)
            pt = ps_misc.tile([1, 1], F32)
            nc.tensor.matmul(pt[:], red[:], ones[:], start=True, stop=True)
            osb = small.tile([1, 1], F32)
            nc.vector.tensor_copy(osb[:], pt[:])
            nc.sync.dma_start(out_d[:], osb[:])

    nc.compile()
    return nc


def get_nc():
    if "nc" not in _CACHE:
        _CACHE["nc"] = _build()
    return _CACHE["nc"]


def make_in_maps(inputs, mps):
    import ml_dtypes

    x = np.asarray(inputs, dtype=np.float32).astype(ml_dtypes.bfloat16)
    mps = np.asarray(mps, dtype=np.float32).reshape(A, E, O, FO)
    # x packed as [e_chunk, e_low, agent]
    x_pack = np.ascontiguousarray(x.reshape(A, 2, 128).transpose(1, 2, 0))
    in_maps = []
    for c in range(N_CORES):
        m = mps[:, :, c, :].astype(ml_dtypes.bfloat16)  # (A, E, FO)
        m = m.reshape(A, 2, 128, FO).transpose(1, 2, 0, 3)  # (2, 128, A, FO)
        in_maps.append(
            {
                "inputs": x_pack,
                "mps": np.ascontiguousarray(m).reshape(2, 128, A * FO),
            }
        )
    return in_maps


def kernel(inputs, mps):
    from concourse.bass_utils import run_bass_kernel_spmd

    nc = get_nc()
    res = run_bass_kernel_spmd(
        nc, make_in_maps(inputs, mps), core_ids=list(range(N_CORES))
    )
    return np.array(
        [res.results[c]["out"][0, 0] for c in range(N_CORES)], dtype=np.float32
    )


# revision 16
# speedup vs baseline: 1.9431x; 1.0021x over previous
"""Trainium2 (8-core SPMD) kernel for the ActorCriticTensorNet MPS head.

reference:
    env0 = einsum('e,eoij->oij', x[0], mps[0])
    for a in 1..63: env = sigmoid(env @ einsum('e,eoij->oij', x[a], mps[a]))
    out = einsum('oii->o', env)

Strategy: the computation factorizes perfectly over the output channel o —
the per-agent contractions mat[a][o] and the 63-step sigmoid chain for
channel o never touch any other channel; the channels only meet in the final
trace vector.  So shard by o: core c receives mps[:, :, c, :, :] (32 MB in
bf16) plus the full (tiny) x, computes all 64 mat[g][c] slices locally,
runs its own o=c chain locally, and writes the single scalar out[c]; the
host assembles the 8 scalars.  Zero inter-core communication, and the chain
consumes mats as phase 1 produces them, so its serial latency hides entirely
under the tensor streaming.

Phase-1 layout (per 8-agent block): weight column (gi, jl) reads the
stride-8 single-free-dim AP f = gi*1024 + 8p + jl (p = 4k + j_hi), giving
psum[p, gi*8+jl] = mat[gi][k, j] (k = p>>2, j = (p&3)*8 + jl).  The
psum->stage copy permutes columns (gi,jl)->(jl,gi) so the DRAM block is
d = 64p + jl*8 + gi = k*256 + j_hi*64 + jl*8 + gi: the store, the one
chain-block load (32 x 256, 512B runs) and the per-agent chain weight views
(stride-8, merging to a single free dim) are all clean APs.

Host-side prep packs shards as mps[c] -> (2, 128, 64*1024) [e-chunk, e_low,
(agent, f)] and x -> (2, 128, 64), so every device DMA is contiguous 2 MB.
All inputs are converted to bf16 on the host (PSUM accumulation is fp32;
measured end-to-end relative error ~6e-4 vs the fp32 reference).
"""

import numpy as np

A, E, O, C = 64, 256, 8, 32
FO = C * C  # per-o mat size: 1024
N_CORES = 8
BLK = 8  # agents per phase-1 block
NBLK = A // BLK

_CACHE = {}


def _build(debug_out=False):
    from concourse import bacc, mybir, tile
    from concourse.masks import make_identity

    F32 = mybir.dt.float32
    BF16 = mybir.dt.bfloat16
    SIG = mybir.ActivationFunctionType.Sigmoid
    nc = bacc.Bacc(
        "TRN2", target_bir_lowering=False, debug=False, num_devices=N_CORES
    )
    x_d = nc.dram_tensor("inputs", [2, 128, A], BF16, kind="ExternalInput")
    mps_d = nc.dram_tensor(
        "mps", [2, 128, A * FO], BF16, kind="ExternalInput"
    )
    out_d = nc.dram_tensor("out", [1, 1], F32, kind="ExternalOutput")
    if debug_out:
        matdbg_d = nc.dram_tensor("matdbg", [A, FO], BF16, kind="ExternalOutput")
        envdbg_d = nc.dram_tensor("envdbg", [C, C], F32, kind="ExternalOutput")

    with tile.TileContext(nc) as tc:
        with (
            tc.tile_pool(name="dram", bufs=1, space="DRAM") as dram,
            tc.tile_pool(name="mps_pool", bufs=9) as mps_pool,
            tc.tile_pool(name="small", bufs=1) as small,
            tc.tile_pool(name="stage_pool", bufs=4) as stage_pool,
            tc.tile_pool(name="chain_pool", bufs=4) as chain_pool,
            tc.tile_pool(name="env_pool", bufs=2) as env_pool,
            tc.tile_pool(name="ps_mat", bufs=3, space="PSUM") as ps_mat,
            tc.tile_pool(name="ps_chain", bufs=2, space="PSUM") as ps_chain,
            tc.tile_pool(name="ps_misc", bufs=1, space="PSUM") as ps_misc,
        ):
            mat_dram = dram.tile([NBLK, 128 * 64], BF16)

            # x_sb[e_lo, eh*64 + g] = x[g, eh*128 + e_lo]
            x_sb = small.tile([128, 2 * A], BF16)
            for eh in range(2):
                nc.sync.dma_start(x_sb[:, eh * A : (eh + 1) * A], x_d[eh])

            ident = small.tile([C, C], BF16)
            make_identity(nc, ident[:])

            envs = [None]


            cvs = {}

            def chain_prefetch(blk):
                # one clean (32, 256) load per block; agent gi's weight is
                # the stride-8 view [jh: 64x4][jl: 8x8] at offset gi, which
                # opt-merges into a single free dim.
                cb = chain_pool.tile([C, 256], BF16, tag="chain", name=f"cb{blk}")
                nc.scalar.dma_start(
                    cb[:],
                    mat_dram[blk : blk + 1, :].rearrange(
                        "u (k r) -> (u k) r", k=C, r=256
                    ),
                )
                cvs[blk] = cb[:].rearrange(
                    "k (jh jl gi) -> k jh jl gi", jh=4, jl=8, gi=8
                )

            def chain_step(g):
                cv, gi = cvs[g // BLK], g % BLK
                init = g == 0
                ps_g = ps_chain.tile([C, C], BF16 if init else F32, tag="cps")
                if init:
                    nc.tensor.transpose(ps_g[:], cv[:, :, :, gi], ident[:])
                else:
                    nc.tensor.matmul(
                        ps_g[:], cv[:, :, :, gi], envs[0][:], start=True, stop=True
                    )
                env2 = env_pool.tile([C, C], BF16, tag="env")
                if init:
                    nc.vector.tensor_copy(env2[:], ps_g[:])
                else:
                    nc.scalar.activation(env2[:], ps_g[:], SIG)
                envs[0] = env2

            # Software-pipelined emission: the PE executes in order, so a
            # chain matmul stalls everything behind it until the previous
            # step's sigmoid lands.  Interleave chain steps one *block
            # behind* phase 1 — by the time the PE reaches a chain matmul,
            # its sigmoid dependency has long retired behind 16 phase-1
            # matmuls.
            for blk in range(NBLK):
                psum_b = ps_mat.tile([128, 64], F32, tag="psa")
                tvs = []
                for eh in range(2):
                    t = mps_pool.tile([128, BLK * FO], BF16, tag="mps")
                    nc.sync.dma_start(
                        t[:],
                        mps_d[eh, :, blk * BLK * FO : (blk + 1) * BLK * FO],
                    )
                    tvs.append(
                        t[:].rearrange(
                            "e (gi p jl) -> e gi p jl", gi=BLK, p=128, jl=8
                        )
                    )
                for gi in range(BLK):
                    g = blk * BLK + gi
                    for jl in range(8):
                        for eh in range(2):
                            nc.tensor.matmul(
                                psum_b[:, gi * 8 + jl : gi * 8 + jl + 1],
                                tvs[eh][:, gi, :, jl],
                                x_sb[:, eh * A + g : eh * A + g + 1],
                                start=(eh == 0),
                                stop=(eh == 1),
                            )
                    if blk >= 1:
                        chain_step((blk - 1) * BLK + gi)
                stage = stage_pool.tile([128, 64], BF16, tag="stage")
                nc.vector.tensor_copy(
                    stage[:].rearrange("p (jl gi) -> p jl gi", jl=8, gi=8),
                    psum_b[:].rearrange("p (gi jl) -> p jl gi", gi=8, jl=8),
                )
                nc.gpsimd.dma_start(mat_dram[blk : blk + 1, :], stage[:])
                chain_prefetch(blk)
            for gi in range(BLK):
                chain_step((NBLK - 1) * BLK + gi)

            env = envs[0]
            if debug_out:
                dbg = small.tile([C, C], F32)
                nc.vector.tensor_copy(dbg[:], env[:])
                nc.sync.dma_start(envdbg_d[:], dbg[:])
                nc.sync.dma_start(
                    matdbg_d[:],
                    mat_dram[:].rearrange("b (p c2) -> (b p) c2", p=128, c2=64),
                )

            # ---- trace epilogue: out = sum_k env_T[k, k] ----
            masked = small.tile([C, C], BF16)
            nc.gpsimd.affine_select(
                out=masked[:],
                in_=env[:],
                compare_op=mybir.AluOpType.is_equal,
                fill=0.0,
                base=0,
                pattern=[[1, C]],
                channel_multiplier=-1,
            )
            red = small.tile([C, 1], F32)
            nc.vector.tensor_reduce(
                red[:],
                masked[:],
                axis=mybir.AxisListType.X,
                op=mybir.AluOpType.add,
            )
            ones = small.tile([C, 1], F32)
            nc.gpsimd.memset(ones[:], 1.0)
            pt = ps_misc.tile([1, 1], F32)
            nc.tensor.matmul(pt[:], red[:], ones[:], start=True, stop=True)
            osb = small.tile([1, 1], F32)
            nc.vector.tensor_copy(osb[:], pt[:])
            nc.sync.dma_start(out_d[:], osb[:])

    nc.compile()
    return nc


def get_nc():
    if "nc" not in _CACHE:
        _CACHE["nc"] = _build()
    return _CACHE["nc"]


def make_in_maps(inputs, mps):
    import ml_dtypes

    x = np.asarray(inputs, dtype=np.float32).astype(ml_dtypes.bfloat16)
    mps = np.asarray(mps, dtype=np.float32).reshape(A, E, O, FO)
    # x packed as [e_chunk, e_low, agent]
    x_pack = np.ascontiguousarray(x.reshape(A, 2, 128).transpose(1, 2, 0))
    in_maps = []
    for c in range(N_CORES):
        m = mps[:, :, c, :].astype(ml_dtypes.bfloat16)  # (A, E, FO)
        m = m.reshape(A, 2, 128, FO).transpose(1, 2, 0, 3)  # (2, 128, A, FO)
        in_maps.append(
            {
                "inputs": x_pack,
                "mps": np.ascontiguousarray(m).reshape(2, 128, A * FO),
            }
        )
    return in_maps


def kernel(inputs, mps):
    from concourse.bass_utils import run_bass_kernel_spmd

    nc = get_nc()
    res = run_bass_kernel_spmd(
        nc, make_in_maps(inputs, mps), core_ids=list(range(N_CORES))
    )
    return np.array(
        [res.results[c]["out"][0, 0] for c in range(N_CORES)], dtype=np.float32
    )
